# revision 1
# baseline (speedup 1.0000x reference)
"""Trainium2 Bass kernel for modulated conv1d (StyleGAN-style Conv1DMod).

Reference computation (per batch sample b):
  wm[k,c,f]  = kern[k,c,f] * coef * (style[b,c] + 1)        (modulate)
  denom[f]   = rsqrt(sum_{k,c} wm[k,c,f]^2)                 (demodulate)
  out[b,f,w] = denom[f] * sum_{k,c} wm[k,c,f] * feat[b,c,w+k-1]   (SAME conv)

Sharding: data-parallel over batch B=8 -> one sample per NeuronCore.
Demodulation is a per-(b,f) linear scale, so it is applied to the conv
*output* tiles (whose partition dim is f) instead of rescaling weights.

The conv runs as 6 PSUM-accumulated fp32r matmuls (single-pass PE fp32)
per [128f, 512w] output tile; fp32r needs producers to round, so the
feature chunks are DMA'd as fp32 and rounded by the otherwise-idle
Scalar engine.
"""

import numpy as np

import concourse.bass as bass
import concourse.mybir as mybir
import concourse.tile as tile

B, C, W, K, F = 8, 256, 8192, 3, 256
COEF = 1.0 / float(np.sqrt(K * C))

P = 128
CT = C // P  # 2 contraction tiles
FT = F // P  # 2 output-partition tiles
WCHUNK = 2048  # X dma chunk width (1 MB per [128, 2048] f32 transfer)
NJ = W // WCHUNK  # 4 chunks
WTILE = 512  # matmul moving-operand width (fp32 max)
NI = WCHUNK // WTILE  # 4 w-tiles per chunk
XCOLS = WCHUNK + 2  # chunk + 1-col halo each side

MAX_WAITS = 1  # walrus codegen in this container rejects >1 sync wait per inst


def _split_sync_waits(nc, limit=MAX_WAITS):
    """Move excess sem-waits onto NoOps inserted before the offending
    instruction (same engine, program order preserved)."""
    uid = 0
    for fn in nc.m.functions:
        for bb in fn.blocks:
            insts = bb.instructions
            changed = False
            newlist = []
            for ins in insts:
                si = ins.sync_info
                if si is not None and len(si.on_wait) > limit:
                    waits = list(si.on_wait)
                    keep = waits[-limit:]
                    excess = waits[:-limit]
                    for k in range(0, len(excess), limit):
                        nop = mybir.InstNoOp(name=f"waitsplit-{uid}", ins=[], outs=[])
                        uid += 1
                        nop.engine = ins.engine
                        nop.sync_info = mybir.SyncInfo(
                            on_wait=excess[k : k + limit], on_update=[]
                        )
                        newlist.append(nop)
                    ins.sync_info = mybir.SyncInfo(
                        on_wait=keep, on_update=list(si.on_update)
                    )
                    changed = True
                newlist.append(ins)
            if changed:
                bb.instructions = newlist


def _conv1dmod_body(tc, feat, style, kern, out):
    nc = tc.nc
    f32 = mybir.dt.float32
    f32r = mybir.dt.float32r

    with (
        tc.tile_pool(name="xbuf", bufs=1) as xbuf,
        tc.tile_pool(name="xraw", bufs=4) as xraw_pool,
        tc.tile_pool(name="wbuf", bufs=1) as wbuf,
        tc.tile_pool(name="stage", bufs=3) as stage_pool,
        tc.tile_pool(name="psum", bufs=7, space="PSUM") as psum_pool,
        tc.tile_pool(name="dpsum", bufs=1, space="PSUM") as dpsum_pool,
    ):
        # ---- small weight DMAs first: they gate every conv matmul. Keep the
        # SP queue free for the feature chunks: style leads on SP (tiny),
        # kern ct0 pieces ride the Scalar HWDGE queue, ct1 pieces the SWDGE
        # queue. kern [K, C, F] flat is [(2K) x 128, F]: piece a=2k+ct is a
        # fully contiguous 128 KB block landing on partitions c%128.
        ssty = wbuf.tile([P, CT], f32, tag="ssty")
        with nc.allow_non_contiguous_dma(reason="256-elem style vector"):
            nc.sync.dma_start(ssty[:], style.rearrange("(o p) -> p o", p=P))
        kflat = kern.rearrange("k (h p) f -> (k h) p f", p=P)
        ksb = [
            wbuf.tile([P, K, F], f32, tag=f"ksb_{ct}", name=f"ksb_{ct}")
            for ct in range(CT)
        ]
        for k in range(K):
            nc.scalar.dma_start(ksb[0][:, k, :], kflat[2 * k])
            nc.gpsimd.dma_start(ksb[1][:, k, :], kflat[2 * k + 1])

        # ---- modulate weights ----
        s1 = wbuf.tile([P, CT], f32, tag="s1")
        nc.vector.tensor_scalar(
            s1[:], ssty[:], 1.0, COEF, mybir.AluOpType.add, mybir.AluOpType.mult
        )
        wm = []
        for ct in range(CT):
            wmt = wbuf.tile([P, K, F], f32r, tag=f"wm_{ct}")
            nc.vector.tensor_scalar_mul(wmt[:], ksb[ct][:], s1[:, ct : ct + 1])
            wm.append(wmt)

        # ---- feature chunks: DMA fp32, round to fp32r on the Scalar
        # engine (it keeps up: ~2.4us/chunk vs the ~4.8us/chunk DMA feed),
        # leaving the DVE free for the weight chain and demod copies.
        def convert(ct, dst, src):
            nc.scalar.copy(dst, src)

        xt = [[None] * NJ for _ in range(CT)]

        def emit_loads(j, npieces=1):
            for ct in range(CT):
                crow = slice(ct * P, (ct + 1) * P)
                t = xbuf.tile([P, XCOLS], f32r, tag=f"x_{ct}_{j}")
                xt[ct][j] = t
                lo = j * WCHUNK - 1
                hi = j * WCHUNK + WCHUNK + 1
                dst_lo = 0
                if lo < 0:
                    nc.vector.memset(t.bitcast(f32)[:, 0:1], 0.0)
                    dst_lo = 1
                    lo = 0
                if hi > W:
                    nc.vector.memset(t.bitcast(f32)[:, XCOLS - 1 : XCOLS], 0.0)
                    hi = W
                bounds = np.linspace(lo, hi, npieces + 1).astype(int)
                for p0, p1 in zip(bounds[:-1], bounds[1:]):
                    ncols = int(p1 - p0)
                    off = dst_lo + int(p0 - lo)
                    raw = xraw_pool.tile([P, XCOLS], f32, tag="xraw")
                    nc.sync.dma_start(raw[:, off : off + ncols], feat[crow, p0:p1])
                    convert(ct, t[:, off : off + ncols], raw[:, off : off + ncols])

        def emit_mms(j, ft):
            """Emit the NI psum accumulation groups for (chunk j, ft)."""
            pss = []
            for i in range(NI):
                ps = psum_pool.tile([P, WTILE], f32, tag="psum")
                first = True
                for ct in range(CT):
                    for k in range(K):
                        nc.tensor.matmul(
                            ps[:],
                            wm[ct][:, k, ft * P : (ft + 1) * P],
                            xt[ct][j][:, i * WTILE + k : i * WTILE + k + WTILE],
                            start=first,
                            stop=(ct == CT - 1 and k == K - 1),
                        )
                        first = False
                pss.append(ps)
            return pss

        def emit_copies(j, ft, pss):
            """Demodulating PSUM->SBUF copies + half-chunk output stores."""
            st = stage_pool.tile([P, WCHUNK], f32, tag="stage")
            for i, ps in enumerate(pss):
                nc.vector.tensor_scalar_mul(
                    st[:, i * WTILE : (i + 1) * WTILE], ps[:], denom[:, ft : ft + 1]
                )
            out_rows = slice(ft * P, (ft + 1) * P)
            # finer stores on the last chunk shorten the end-of-kernel tail
            npieces = 4 if j == NJ - 1 else 2
            piece = WCHUNK // npieces
            for h in range(npieces):
                out_cols = slice(j * WCHUNK + h * piece, j * WCHUNK + (h + 1) * piece)
                nc.sync.dma_start(
                    out[out_rows, out_cols], st[:, h * piece : (h + 1) * piece]
                )

        # chunk-0 loads + its first matmul block go ahead of everything else
        emit_loads(0, npieces=2)
        pss00 = emit_mms(0, 0)

        # ---- demodulation scale: denom[f] = rsqrt(sum_{k,c} wm^2) ----
        # Emitted after the first conv block so the tiny demod matmuls do
        # not sit at the head of the in-order PE queue waiting on the DVE
        # square/sum chain.
        ssq = []
        for ct in range(CT):
            sqt = wbuf.tile([P, K, F], f32, tag=f"sq_{ct}")
            nc.vector.tensor_mul(sqt[:], wm[ct].bitcast(f32)[:], wm[ct].bitcast(f32)[:])
            sst = wbuf.tile([P, F], f32, tag=f"ssq_{ct}")
            nc.vector.tensor_add(sst[:], sqt[:, 0], sqt[:, 1])
            nc.vector.tensor_add(sst[:], sst[:], sqt[:, 2])
            ssq.append(sst)
        ones = wbuf.tile([P, 1], f32, tag="ones")
        nc.vector.memset(ones[:], 1.0)
        dp = dpsum_pool.tile([P, FT], f32, tag="dpsum")
        for ft in range(FT):
            for ct in range(CT):
                nc.tensor.matmul(
                    dp[:, ft : ft + 1],
                    ssq[ct][:, ft * P : (ft + 1) * P],
                    ones[:],
                    start=(ct == 0),
                    stop=(ct == CT - 1),
                )
        denom = wbuf.tile([P, FT], f32, tag="denom")
        nc.scalar.activation(denom[:], dp[:], mybir.ActivationFunctionType.Sqrt)
        nc.vector.reciprocal(denom[:], denom[:])

        # ---- conv: chunk loads stay one chunk ahead of the matmul stream ----
        emit_loads(1)
        emit_copies(0, 0, pss00)
        emit_copies(0, 1, emit_mms(0, 1))
        for j in range(1, NJ):
            if j + 1 < NJ:
                emit_loads(j + 1)
            for ft in range(FT):
                emit_copies(j, ft, emit_mms(j, ft))


def build_bass():
    nc = bass.Bass(name="conv1dmod")
    feat = nc.dram_tensor("feature", [C, W], mybir.dt.float32, kind="ExternalInput")
    style = nc.dram_tensor("style", [C], mybir.dt.float32, kind="ExternalInput")
    kern = nc.dram_tensor("kern", [K, C, F], mybir.dt.float32, kind="ExternalInput")
    out = nc.dram_tensor("out", [F, W], mybir.dt.float32, kind="ExternalOutput")
    with tile.TileContext(nc) as tc:
        _conv1dmod_body(tc, feat, style, kern, out)
    _split_sync_waits(nc)
    return nc


_NC_CACHE = None


def kernel(feature, style, kernel):
    """Full-input entry point: shard over batch across 8 cores, run, gather."""
    global _NC_CACHE
    from concourse.bass_utils import run_bass_kernel_spmd

    if _NC_CACHE is None:
        _NC_CACHE = build_bass()
    nc = _NC_CACHE

    feature = np.ascontiguousarray(feature, dtype=np.float32)
    style = np.ascontiguousarray(style, dtype=np.float32)
    kernel = np.ascontiguousarray(kernel, dtype=np.float32)

    in_maps = [
        {"feature": feature[b], "style": style[b], "kern": kernel} for b in range(B)
    ]
    res = run_bass_kernel_spmd(nc, in_maps, core_ids=list(range(B)))
    return np.stack([r["out"] for r in res.results], axis=0)



# revision 9
# speedup vs baseline: 1.0082x; 1.0082x over previous
"""Trainium2 Bass kernel for modulated conv1d (StyleGAN-style Conv1DMod).

Reference computation (per batch sample b):
  wm[k,c,f]  = kern[k,c,f] * coef * (style[b,c] + 1)        (modulate)
  denom[f]   = rsqrt(sum_{k,c} wm[k,c,f]^2)                 (demodulate)
  out[b,f,w] = denom[f] * sum_{k,c} wm[k,c,f] * feat[b,c,w+k-1]   (SAME conv)

Sharding: data-parallel over batch B=8 -> one sample per NeuronCore.

Schedule notes (v2):
 - Contraction c is mapped to (partition p, group h) as c = 2p + h. This
   makes each kern[k] piece a [128, 2x256] tile whose per-partition source
   run is 2 KB contiguous (vs 1 KB for the c = ct*128+p blocking), halving
   descriptor count for the weight DMAs that gate the first matmul.
 - kern k-pieces ride three queues in parallel (SP / Activation HWDGE /
   Pool SWDGE); style leads on SP.
 - Feature chunks DMA as fp32 and are rounded to fp32r (PE single-pass
   fp32) by Scalar; the first two chunk-0 pieces round on DVE so the PE
   stream can start while Scalar still holds the kern[1] DMA issue.
 - All feature loads and output stores share the SP queue in priority
   order; Scalar does rounding only, DVE does modulate + demod copies.
 - Demod scale is applied on the conv output tiles (partition dim = f).
"""

import numpy as np

import concourse.bass as bass
import concourse.mybir as mybir
import concourse.tile as tile

B, C, W, K, F = 8, 256, 8192, 3, 256
COEF = 1.0 / float(np.sqrt(K * C))

P = 128
H = 2  # contraction groups: c = 2*p + h
FT = F // P  # 2 output-partition tiles
WCHUNK = 2048
NJ = W // WCHUNK  # 4 chunks
WTILE = 512  # matmul moving-operand width (PSUM bank = 512 f32)
NI = WCHUNK // WTILE  # 4 w-tiles per chunk
XCOLS = WCHUNK + 2  # chunk + 1-col halo each side

MAX_WAITS = 1  # walrus codegen in this container rejects >1 sync wait per inst


def _split_sync_waits(nc, limit=MAX_WAITS):
    """Move excess sem-waits onto NoOps inserted before the offending
    instruction (same engine, program order preserved)."""
    uid = 0
    for fn in nc.m.functions:
        for bb in fn.blocks:
            insts = bb.instructions
            changed = False
            newlist = []
            for ins in insts:
                si = ins.sync_info
                if si is not None and len(si.on_wait) > limit:
                    waits = list(si.on_wait)
                    keep = waits[-limit:]
                    excess = waits[:-limit]
                    for k in range(0, len(excess), limit):
                        nop = mybir.InstNoOp(name=f"waitsplit-{uid}", ins=[], outs=[])
                        uid += 1
                        nop.engine = ins.engine
                        nop.sync_info = mybir.SyncInfo(
                            on_wait=excess[k : k + limit], on_update=[]
                        )
                        newlist.append(nop)
                    ins.sync_info = mybir.SyncInfo(
                        on_wait=keep, on_update=list(si.on_update)
                    )
                    changed = True
                newlist.append(ins)
            if changed:
                bb.instructions = newlist


def _conv1dmod_body(tc, feat, style, kern, out):
    nc = tc.nc
    f32 = mybir.dt.float32
    f32r = mybir.dt.float32r
    add = mybir.AluOpType.add
    mult = mybir.AluOpType.mult

    featr = feat.rearrange("(p h) w -> p h w", h=H)
    # kern [K, C, F] -> [k, p, (h f)]: per (k, p) the (h f) run is 2 KB contiguous
    ksrc = kern.rearrange("k (p h) f -> k p (h f)", h=H)

    with (
        tc.tile_pool(name="wbuf", bufs=1) as wbuf,
        tc.tile_pool(name="xbuf", bufs=1) as xbuf,
        tc.tile_pool(name="xraw", bufs=6) as xraw_pool,
        tc.tile_pool(name="stage", bufs=4) as stage_pool,
        tc.tile_pool(name="psum", bufs=7, space="PSUM") as psum_pool,
        tc.tile_pool(name="dpsum", bufs=1, space="PSUM") as dpsum_pool,
    ):
        # ---- head DMAs: style + kern[0] on SP, kern[1] on Activation,
        # kern[2] on Pool SWDGE; every kern piece is 128 x 2KB descriptors.
        ssty = wbuf.tile([P, H], f32, tag="ssty")
        with nc.allow_non_contiguous_dma(reason="256-elem style vector"):
            nc.sync.dma_start(ssty[:], style.rearrange("(p h) -> p h", h=H))
        kt = [
            wbuf.tile([P, H * F], f32, tag=f"kt{k}", name=f"kt{k}") for k in range(K)
        ]
        nc.sync.dma_start(kt[0][:], ksrc[0])
        nc.scalar.dma_start(kt[1][:], ksrc[1])
        nc.gpsimd.dma_start(kt[2][:], ksrc[2])

        # warm the Scalar activation table (Sqrt) off the critical path
        warm = wbuf.tile([P, 1], f32, tag="warm")
        nc.vector.memset(warm[:], 1.0)
        warm2 = wbuf.tile([P, 1], f32, tag="warm2")
        nc.scalar.sqrt(warm2[:], warm[:])

        ones = wbuf.tile([P, 1], f32, tag="ones")
        nc.vector.memset(ones[:], 1.0)

        # ---- x tiles (fp32r) + raw fp32 staging ----
        xt = [[None] * NJ for _ in range(H)]
        for h in range(H):
            for j in range(NJ):
                xt[h][j] = xbuf.tile(
                    [P, XCOLS], f32r, tag=f"x_{h}_{j}", name=f"x_{h}_{j}"
                )
        # halo edges
        nc.vector.memset(xt[0][0].bitcast(f32)[:, 0:1], 0.0)
        nc.vector.memset(xt[1][0].bitcast(f32)[:, 0:1], 0.0)
        nc.vector.memset(xt[0][NJ - 1].bitcast(f32)[:, XCOLS - 1 : XCOLS], 0.0)
        nc.vector.memset(xt[1][NJ - 1].bitcast(f32)[:, XCOLS - 1 : XCOLS], 0.0)

        def load_piece(j, h, c0, c1):
            """DMA tile cols [c0,c1) of chunk j / group h; return convert args.

            Tile col c holds feat col j*WCHUNK - 1 + c (halo offset).
            """
            lo = j * WCHUNK - 1 + c0
            hi = j * WCHUNK - 1 + c1
            lo = max(lo, 0)
            hi = min(hi, W)
            d0 = lo - (j * WCHUNK - 1)  # dst col of first loaded element
            ncols = hi - lo
            raw = xraw_pool.tile([P, ncols], f32, tag="xraw")
            nc.sync.dma_start(raw[:], featr[:, h, lo:hi])
            return (xt[h][j][:, d0 : d0 + ncols], raw[:])

        def cvt_scalar(dst, src):
            nc.scalar.copy(dst, src)

        def cvt_vector(dst, src):
            nc.vector.tensor_scalar_add(dst, src, 0.0)

        # ---- modulate helpers ----
        s1 = wbuf.tile([P, H], f32, tag="s1")
        wm = wbuf.tile([P, K, H * F], f32r, tag="wm")

        def emit_s1():
            nc.vector.tensor_scalar(s1[:], ssty[:], 1.0, COEF, add, mult)

        def emit_mod(k, h):
            nc.vector.tensor_scalar_mul(
                wm[:, k, h * F : (h + 1) * F],
                kt[k][:, h * F : (h + 1) * F],
                s1[:, h : h + 1],
            )

        # ---- chunk-0 loads: 4 pieces per group, h-interleaved; first piece
        # of each group converts on DVE (Scalar is busy issuing kern[1]).
        c0_bounds = [0, 518, 1030, 1542, 2050]
        j0_cvts = []
        for pc in range(4):
            for h in range(H):
                j0_cvts.append(load_piece(0, h, c0_bounds[pc], c0_bounds[pc + 1]))

        emit_s1()
        emit_mod(0, 0)
        emit_mod(0, 1)
        cvt_vector(*j0_cvts[0])
        cvt_vector(*j0_cvts[1])
        emit_mod(1, 0)
        emit_mod(1, 1)
        emit_mod(2, 0)
        emit_mod(2, 1)
        for args in j0_cvts[2:]:
            cvt_scalar(*args)

        # ---- demod inputs: ssq[p, f] = sum_k sum_h wm^2 ----
        wmf = wm.bitcast(f32)
        sq = wbuf.tile([P, K, H * F], f32, tag="sq")
        nc.vector.tensor_mul(sq[:], wmf[:], wmf[:])
        acc = wbuf.tile([P, H * F], f32, tag="acc")
        nc.vector.tensor_add(acc[:], sq[:, 0], sq[:, 1])
        nc.vector.tensor_add(acc[:], acc[:], sq[:, 2])
        ssq = wbuf.tile([P, F], f32, tag="ssq")
        nc.vector.tensor_add(ssq[:], acc[:, 0:F], acc[:, F : 2 * F])

        def emit_group(j, ft, i):
            """6 PSUM-accumulated matmuls for output tile (j, ft, i)."""
            ps = psum_pool.tile([P, WTILE], f32, tag="ps")
            first = True
            for k in range(K):
                for h in range(H):
                    nc.tensor.matmul(
                        ps[:],
                        wm[:, k, h * F + ft * P : h * F + ft * P + P],
                        xt[h][j][:, i * WTILE + k : i * WTILE + k + WTILE],
                        start=first,
                        stop=(k == K - 1 and h == H - 1),
                    )
                    first = False
            return ps

        # chunk-0 / ft=0 groups lead the PE queue
        pss00 = [emit_group(0, 0, i) for i in range(NI)]  # noqa

        # ---- denom[f'] = rsqrt(sum_p ssq) via two 1-col matmuls ----
        dp = dpsum_pool.tile([P, FT], f32, tag="dpsum")
        for ft in range(FT):
            nc.tensor.matmul(
                dp[:, ft : ft + 1],
                ssq[:, ft * P : (ft + 1) * P],
                ones[:],
                start=True,
                stop=True,
            )
        denom = wbuf.tile([P, FT], f32, tag="denom")
        nc.scalar.sqrt(denom[:], dp[:])
        nc.vector.reciprocal(denom[:], denom[:])

        def emit_copies(j, ft, pss):
            st = stage_pool.tile([P, WCHUNK], f32, tag="stage")
            for i, ps in enumerate(pss):
                nc.vector.tensor_scalar_mul(
                    st[:, i * WTILE : (i + 1) * WTILE], ps[:], denom[:, ft : ft + 1]
                )
            return st

        def emit_stores(j, ft, st, npieces):
            out_rows = slice(ft * P, (ft + 1) * P)
            piece = WCHUNK // npieces
            for g in range(npieces):
                out_cols = slice(j * WCHUNK + g * piece, j * WCHUNK + (g + 1) * piece)
                nc.sync.dma_start(
                    out[out_rows, out_cols], st[:, g * piece : (g + 1) * piece]
                )

        def load_chunk(j):
            """2 pieces per group, h-interleaved; converts on Scalar."""
            bounds = [0, 1026, 2050]
            for pc in range(2):
                args = [load_piece(j, h, bounds[pc], bounds[pc + 1]) for h in range(H)]
                for a in args:
                    cvt_scalar(*a)

        # ---- steady state ----
        load_chunk(1)
        st = emit_copies(0, 0, pss00)
        pss01 = [emit_group(0, 1, i) for i in range(NI)]
        load_chunk(2)
        emit_stores(0, 0, st, 2)
        st = emit_copies(0, 1, pss01)
        pss10 = [emit_group(1, 0, i) for i in range(NI)]
        emit_stores(0, 1, st, 2)
        st = emit_copies(1, 0, pss10)
        pss11 = [emit_group(1, 1, i) for i in range(NI)]
        load_chunk(3)
        emit_stores(1, 0, st, 2)
        st = emit_copies(1, 1, pss11)
        pss20 = [emit_group(2, 0, i) for i in range(NI)]
        emit_stores(1, 1, st, 2)
        st = emit_copies(2, 0, pss20)
        pss21 = [emit_group(2, 1, i) for i in range(NI)]
        emit_stores(2, 0, st, 2)
        st = emit_copies(2, 1, pss21)
        emit_stores(2, 1, st, 2)
        # last chunk: per-tile copy + store to shorten the tail
        for ft in range(FT):
            pss = [emit_group(3, ft, i) for i in range(NI)]
            st = stage_pool.tile([P, WCHUNK], f32, tag="stage")
            out_rows = slice(ft * P, (ft + 1) * P)
            for i, ps in enumerate(pss):
                nc.vector.tensor_scalar_mul(
                    st[:, i * WTILE : (i + 1) * WTILE], ps[:], denom[:, ft : ft + 1]
                )
                out_cols = slice(3 * WCHUNK + i * WTILE, 3 * WCHUNK + (i + 1) * WTILE)
                nc.sync.dma_start(
                    out[out_rows, out_cols], st[:, i * WTILE : (i + 1) * WTILE]
                )


def build_bass():
    nc = bass.Bass(name="conv1dmod")
    feat = nc.dram_tensor("feature", [C, W], mybir.dt.float32, kind="ExternalInput")
    style = nc.dram_tensor("style", [C], mybir.dt.float32, kind="ExternalInput")
    kern = nc.dram_tensor("kern", [K, C, F], mybir.dt.float32, kind="ExternalInput")
    out = nc.dram_tensor("out", [F, W], mybir.dt.float32, kind="ExternalOutput")
    with tile.TileContext(nc) as tc:
        _conv1dmod_body(tc, feat, style, kern, out)
    _split_sync_waits(nc)
    return nc


_NC_CACHE = None


def kernel(feature, style, kernel):
    """Full-input entry point: shard over batch across 8 cores, run, gather."""
    global _NC_CACHE
    from concourse.bass_utils import run_bass_kernel_spmd

    if _NC_CACHE is None:
        _NC_CACHE = build_bass()
    nc = _NC_CACHE

    feature = np.ascontiguousarray(feature, dtype=np.float32)
    style = np.ascontiguousarray(style, dtype=np.float32)
    kernel = np.ascontiguousarray(kernel, dtype=np.float32)

    in_maps = [
        {"feature": feature[b], "style": style[b], "kern": kernel} for b in range(B)
    ]
    res = run_bass_kernel_spmd(nc, in_maps, core_ids=list(range(B)))
    return np.stack([r["out"] for r in res.results], axis=0)


# revision 10
# speedup vs baseline: 1.0587x; 1.0501x over previous
"""Trainium2 Bass kernel for modulated conv1d (StyleGAN-style Conv1DMod).

Reference computation (per batch sample b):
  wm[k,c,f]  = kern[k,c,f] * coef * (style[b,c] + 1)        (modulate)
  denom[f]   = rsqrt(sum_{k,c} wm[k,c,f]^2)                 (demodulate)
  out[b,f,w] = denom[f] * sum_{k,c} wm[k,c,f] * feat[b,c,w+k-1]   (SAME conv)

Sharding: data-parallel over batch B=8 -> one sample per NeuronCore.

Schedule notes (v3):
 - PE runs in bf16 (tolerance is 2e-2; bf16 conv lands ~2e-3): LDWEIGHTS
   is half the fp32r size so the per-matmul weight reload hides under the
   previous matmul's drain, and the fp32->bf16 rounding passes are 2x
   cheaper than fp32->fp32r.
 - Contraction c is mapped to (partition p, group h) as c = 2p + h so each
   kern[k] piece is a [128, 2x256] tile with 2 KB contiguous descriptors.
 - Each dma_start costs ~0.6-1.1us of sequencer issue time, so transfers
   are few and large: kern rides Scalar(x2)/Pool(x1) queues, style leads
   on Scalar; SP carries all feature loads then output stores in priority
   order (loads first; stores slotted into the slack).
 - Demod scale is applied on the conv output tiles (partition dim = f).
"""

import numpy as np

import concourse.bass as bass
import concourse.mybir as mybir
import concourse.tile as tile

B, C, W, K, F = 8, 256, 8192, 3, 256
COEF = 1.0 / float(np.sqrt(K * C))

P = 128
H = 2  # contraction groups: c = 2*p + h
FT = F // P  # 2 output-partition tiles
WCHUNK = 2048
NJ = W // WCHUNK  # 4 chunks
WTILE = 512  # matmul moving-operand width (PSUM bank = 512 f32)
NI = WCHUNK // WTILE  # 4 w-tiles per chunk
XCOLS = WCHUNK + 2  # chunk + 1-col halo each side

MAX_WAITS = 1  # walrus codegen in this container rejects >1 sync wait per inst


def _split_sync_waits(nc, limit=MAX_WAITS):
    """Move excess sem-waits onto NoOps inserted before the offending
    instruction (same engine, program order preserved)."""
    uid = 0
    for fn in nc.m.functions:
        for bb in fn.blocks:
            insts = bb.instructions
            changed = False
            newlist = []
            for ins in insts:
                si = ins.sync_info
                if si is not None and len(si.on_wait) > limit:
                    waits = list(si.on_wait)
                    keep = waits[-limit:]
                    excess = waits[:-limit]
                    for k in range(0, len(excess), limit):
                        nop = mybir.InstNoOp(name=f"waitsplit-{uid}", ins=[], outs=[])
                        uid += 1
                        nop.engine = ins.engine
                        nop.sync_info = mybir.SyncInfo(
                            on_wait=excess[k : k + limit], on_update=[]
                        )
                        newlist.append(nop)
                    ins.sync_info = mybir.SyncInfo(
                        on_wait=keep, on_update=list(si.on_update)
                    )
                    changed = True
                newlist.append(ins)
            if changed:
                bb.instructions = newlist


def _conv1dmod_body(tc, feat, style, kern, out):
    nc = tc.nc
    f32 = mybir.dt.float32
    bf16 = mybir.dt.bfloat16
    add = mybir.AluOpType.add
    mult = mybir.AluOpType.mult

    featr = feat.rearrange("(p h) w -> p h w", h=H)
    # kern [K, C, F] -> [k, p, (h f)]: per (k, p) the (h f) run is 2 KB contiguous
    ksrc = kern.rearrange("k (p h) f -> k p (h f)", h=H)

    with (
        tc.tile_pool(name="wbuf", bufs=1) as wbuf,
        tc.tile_pool(name="xbuf", bufs=1) as xbuf,
        tc.tile_pool(name="xraw", bufs=6) as xraw_pool,
        tc.tile_pool(name="stage", bufs=4) as stage_pool,
        tc.tile_pool(name="psum", bufs=7, space="PSUM") as psum_pool,
        tc.tile_pool(name="dpsum", bufs=1, space="PSUM") as dpsum_pool,
    ):
        # ---- head DMAs: style + kern[0..1] on Activation, kern[2] on Pool
        # SWDGE; SP starts straight on the feature pieces.
        ssty = wbuf.tile([P, H], f32, tag="ssty")
        with nc.allow_non_contiguous_dma(reason="256-elem style vector"):
            nc.scalar.dma_start(ssty[:], style.rearrange("(p h) -> p h", h=H))
        kt = [
            wbuf.tile([P, H * F], f32, tag=f"kt{k}", name=f"kt{k}") for k in range(K)
        ]
        nc.scalar.dma_start(kt[0][:], ksrc[0])
        nc.scalar.dma_start(kt[1][:], ksrc[1])
        nc.gpsimd.dma_start(kt[2][:], ksrc[2])

        # warm the Scalar activation table (Sqrt) off the critical path
        warm = wbuf.tile([P, 1], f32, tag="warm")
        nc.vector.memset(warm[:], 1.0)
        warm2 = wbuf.tile([P, 1], f32, tag="warm2")
        nc.scalar.sqrt(warm2[:], warm[:])

        ones = wbuf.tile([P, 1], f32, tag="ones")
        nc.vector.memset(ones[:], 1.0)

        # ---- x tiles (bf16) + raw fp32 staging ----
        xt = [[None] * NJ for _ in range(H)]
        for h in range(H):
            for j in range(NJ):
                xt[h][j] = xbuf.tile(
                    [P, XCOLS], bf16, tag=f"x_{h}_{j}", name=f"x_{h}_{j}"
                )
        # halo edges
        nc.vector.memset(xt[0][0][:, 0:1], 0.0)
        nc.vector.memset(xt[1][0][:, 0:1], 0.0)
        nc.vector.memset(xt[0][NJ - 1][:, XCOLS - 1 : XCOLS], 0.0)
        nc.vector.memset(xt[1][NJ - 1][:, XCOLS - 1 : XCOLS], 0.0)

        def load_piece(j, h, c0, c1):
            """DMA tile cols [c0,c1) of chunk j / group h; return convert args.

            Tile col c holds feat col j*WCHUNK - 1 + c (halo offset).
            """
            lo = j * WCHUNK - 1 + c0
            hi = j * WCHUNK - 1 + c1
            lo = max(lo, 0)
            hi = min(hi, W)
            d0 = lo - (j * WCHUNK - 1)  # dst col of first loaded element
            ncols = hi - lo
            raw = xraw_pool.tile([P, ncols], f32, tag="xraw")
            nc.sync.dma_start(raw[:], featr[:, h, lo:hi])
            return (xt[h][j][:, d0 : d0 + ncols], raw[:])

        def cvt_scalar(dst, src):
            nc.scalar.copy(dst, src)

        def cvt_vector(dst, src):
            nc.vector.tensor_scalar_add(dst, src, 0.0)

        # ---- modulate (bf16 weights) ----
        s1 = wbuf.tile([P, H], f32, tag="s1")
        wm = wbuf.tile([P, K, H * F], bf16, tag="wm")

        def emit_s1():
            nc.vector.tensor_scalar(s1[:], ssty[:], 1.0, COEF, add, mult)

        def emit_mod(k, h):
            nc.vector.tensor_scalar_mul(
                wm[:, k, h * F : (h + 1) * F],
                kt[k][:, h * F : (h + 1) * F],
                s1[:, h : h + 1],
            )

        # ---- chunk-0 loads: 3 pieces per group, h-interleaved; h=0 pieces
        # convert on DVE (Scalar still holds the kern issues), h=1 on Scalar.
        c0_bounds = [0, 518, 1030, 2050]
        j0_cvt_v = []  # DVE converts (h=0)
        j0_cvt_s = []  # Scalar converts (h=1)
        for pc in range(3):
            j0_cvt_v.append(load_piece(0, 0, c0_bounds[pc], c0_bounds[pc + 1]))
            j0_cvt_s.append(load_piece(0, 1, c0_bounds[pc], c0_bounds[pc + 1]))

        emit_s1()
        emit_mod(0, 0)
        emit_mod(0, 1)
        cvt_vector(*j0_cvt_v[0])
        cvt_scalar(*j0_cvt_s[0])
        emit_mod(1, 0)
        emit_mod(1, 1)
        emit_mod(2, 0)
        emit_mod(2, 1)
        cvt_vector(*j0_cvt_v[1])
        cvt_scalar(*j0_cvt_s[1])
        cvt_vector(*j0_cvt_v[2])
        cvt_scalar(*j0_cvt_s[2])

        # ---- demod inputs: ssq[p, f] = sum_k sum_h wm^2 ----
        sq = wbuf.tile([P, K, H * F], f32, tag="sq")
        nc.vector.tensor_mul(sq[:], wm[:], wm[:])
        acc = wbuf.tile([P, H * F], f32, tag="acc")
        nc.vector.tensor_add(acc[:], sq[:, 0], sq[:, 1])
        nc.vector.tensor_add(acc[:], acc[:], sq[:, 2])
        ssq = wbuf.tile([P, F], f32, tag="ssq")
        nc.vector.tensor_add(ssq[:], acc[:, 0:F], acc[:, F : 2 * F])

        def emit_group(j, ft, i):
            """6 PSUM-accumulated matmuls for output tile (j, ft, i)."""
            ps = psum_pool.tile([P, WTILE], f32, tag="ps")
            first = True
            for h in range(H):
                for k in range(K):
                    nc.tensor.matmul(
                        ps[:],
                        wm[:, k, h * F + ft * P : h * F + ft * P + P],
                        xt[h][j][:, i * WTILE + k : i * WTILE + k + WTILE],
                        start=first,
                        stop=(k == K - 1 and h == H - 1),
                    )
                    first = False
            return ps

        # chunk-0 / ft=0 groups lead the PE queue
        pss00 = [emit_group(0, 0, i) for i in range(NI)]

        # ---- denom[f'] = rsqrt(sum_p ssq) via two 1-col matmuls ----
        dp = dpsum_pool.tile([P, FT], f32, tag="dpsum")
        for ft in range(FT):
            nc.tensor.matmul(
                dp[:, ft : ft + 1],
                ssq[:, ft * P : (ft + 1) * P],
                ones[:],
                start=True,
                stop=True,
            )
        denom = wbuf.tile([P, FT], f32, tag="denom")
        nc.scalar.sqrt(denom[:], dp[:])
        nc.vector.reciprocal(denom[:], denom[:])

        def emit_copies(j, ft, pss):
            st = stage_pool.tile([P, WCHUNK], f32, tag="stage")
            for i, ps in enumerate(pss):
                nc.vector.tensor_scalar_mul(
                    st[:, i * WTILE : (i + 1) * WTILE], ps[:], denom[:, ft : ft + 1]
                )
            return st

        def emit_stores(j, ft, st, npieces):
            out_rows = slice(ft * P, (ft + 1) * P)
            piece = WCHUNK // npieces
            for g in range(npieces):
                out_cols = slice(j * WCHUNK + g * piece, j * WCHUNK + (g + 1) * piece)
                nc.sync.dma_start(
                    out[out_rows, out_cols], st[:, g * piece : (g + 1) * piece]
                )

        def load_chunk(j):
            """2 pieces per group, h-interleaved; converts on Scalar."""
            bounds = [0, 1026, 2050]
            for pc in range(2):
                args = [load_piece(j, h, bounds[pc], bounds[pc + 1]) for h in range(H)]
                for a in args:
                    cvt_scalar(*a)

        # ---- steady state ----
        load_chunk(1)
        st = emit_copies(0, 0, pss00)
        pss01 = [emit_group(0, 1, i) for i in range(NI)]
        load_chunk(2)
        emit_stores(0, 0, st, 2)
        st = emit_copies(0, 1, pss01)
        pss10 = [emit_group(1, 0, i) for i in range(NI)]
        emit_stores(0, 1, st, 2)
        st = emit_copies(1, 0, pss10)
        pss11 = [emit_group(1, 1, i) for i in range(NI)]
        load_chunk(3)
        emit_stores(1, 0, st, 2)
        st = emit_copies(1, 1, pss11)
        pss20 = [emit_group(2, 0, i) for i in range(NI)]
        emit_stores(1, 1, st, 2)
        st = emit_copies(2, 0, pss20)
        pss21 = [emit_group(2, 1, i) for i in range(NI)]
        emit_stores(2, 0, st, 2)
        st = emit_copies(2, 1, pss21)
        emit_stores(2, 1, st, 2)
        # last chunk: per-tile copy + store to shorten the tail
        for ft in range(FT):
            pss = [emit_group(3, ft, i) for i in range(NI)]
            st = stage_pool.tile([P, WCHUNK], f32, tag="stage")
            out_rows = slice(ft * P, (ft + 1) * P)
            for i, ps in enumerate(pss):
                nc.vector.tensor_scalar_mul(
                    st[:, i * WTILE : (i + 1) * WTILE], ps[:], denom[:, ft : ft + 1]
                )
                out_cols = slice(3 * WCHUNK + i * WTILE, 3 * WCHUNK + (i + 1) * WTILE)
                nc.sync.dma_start(
                    out[out_rows, out_cols], st[:, i * WTILE : (i + 1) * WTILE]
                )


def build_bass():
    nc = bass.Bass(name="conv1dmod")
    feat = nc.dram_tensor("feature", [C, W], mybir.dt.float32, kind="ExternalInput")
    style = nc.dram_tensor("style", [C], mybir.dt.float32, kind="ExternalInput")
    kern = nc.dram_tensor("kern", [K, C, F], mybir.dt.float32, kind="ExternalInput")
    out = nc.dram_tensor("out", [F, W], mybir.dt.float32, kind="ExternalOutput")
    with tile.TileContext(nc) as tc:
        _conv1dmod_body(tc, feat, style, kern, out)
    _split_sync_waits(nc)
    return nc


_NC_CACHE = None


def kernel(feature, style, kernel):
    """Full-input entry point: shard over batch across 8 cores, run, gather."""
    global _NC_CACHE
    from concourse.bass_utils import run_bass_kernel_spmd

    if _NC_CACHE is None:
        _NC_CACHE = build_bass()
    nc = _NC_CACHE

    feature = np.ascontiguousarray(feature, dtype=np.float32)
    style = np.ascontiguousarray(style, dtype=np.float32)
    kernel = np.ascontiguousarray(kernel, dtype=np.float32)

    in_maps = [
        {"feature": feature[b], "style": style[b], "kern": kernel} for b in range(B)
    ]
    res = run_bass_kernel_spmd(nc, in_maps, core_ids=list(range(B)))
    return np.stack([r["out"] for r in res.results], axis=0)
